# revision 3
# baseline (speedup 1.0000x reference)
"""Trainium2 Bass kernel for nn_CSA_ConvBlock (conv-self-attention block).

Reference math (B,C,H,W = 16,256,64,64):
  fq = conv3x3(x, wq); fk = conv3x3(x, wk); fv = conv3x3(x, wv)
  q_sum = fq.sum(H); k_sum = fk.sum(C,H)
  f_scores[b,c] = sum_w q_sum[b,c,w]*k_sum[b,w] / (sqrt(W)*H^2)
  scores = softmax_C(f_scores)
  out = relu(BN_eval(scores*fv + x))

Key algebraic reduction: fq and fk are only consumed through H-sums, and
conv is linear, so q_sum/k_sum collapse to 3-tap-x-3-dy matmuls over the
column sums of x (with top/bottom row edge corrections for SAME padding).
Only conv(x, wv) is computed in full.

The fv conv runs in fp8e4 DoubleRow mode: each matmul contracts BOTH
128-channel k-tiles at once (lhsT [128,2,128], rhs [128,2,512]), halving
the PE LDWEIGHTS+MATMUL pair count vs bf16 (the baseline was bound on
per-pair weight-load overhead, not matmul streaming).  The host ships the
image as three dx-pre-shifted zero-padded fp8 planes with contiguous
64-wide rows, so every conv matmul's moving operand is a contiguous
[128, 2, 512] slice; the dy tap shift is just a row offset.  Weights are
scaled by 64 into fp8's normal range; the 1/64 is folded into the
per-channel eviction scale.  Since scores ~ 1/C ~ 0.004, the attention
branch is strongly suppressed relative to the residual x, so the fp8 conv
and a bf16 residual are numerically safe (rel err << 2e-2 gate).

Sharding: data-parallel over batch, 2 batches per core on 8 cores.
"""

import os
import sys
import numpy as np
from contextlib import ExitStack

if "/opt/trn_rl_repo" not in sys.path and not any(
    "trn_rl_repo" in p for p in sys.path
):
    sys.path.insert(0, "/opt/trn_rl_repo")

import concourse.bass as bass
import concourse.tile as tile
from concourse import bacc, mybir
from concourse import bass_utils

B, C, H, W = 16, 256, 64, 64
NCORES = 8
BPC = B // NCORES          # batches per core
P = 128                    # partitions
KT = C // P                # channel k-tiles (2)
MT = C // P                # channel m-tiles (2)
PW = W + 2                 # padded width 66
PH = H + 2                 # padded height 66
NDX = 3                    # dx-shifted plane copies
KTP = PH * W               # kt stride within a dx plane (4224 B, 16-aligned)
DXP = KT * KTP             # dx plane-pair stride (8448 B)
NTAP = 9
ROWS_PER_ST = 8
NF = ROWS_PER_ST * W       # 512 free elems per spatial tile
ST = (H * W) // NF         # 8 spatial tiles per (batch, mtile)
EPS = 1e-5
WSCALE = 64.0              # fp8 weight pre-scale (folded out at eviction)
SCORE_SCALE = 1.0 / (np.sqrt(np.float32(W)) * (H * H))  # 1/32768

FP32 = mybir.dt.float32
BF16 = mybir.dt.bfloat16
FP8 = mybir.dt.float8e4
AX = mybir.AxisListType
ALU = mybir.AluOpType
ACTF = mybir.ActivationFunctionType
DR = mybir.MatmulPerfMode.DoubleRow

# tap order: dx=1 plane first (arrives first; also feeds the colsums)
TAP_ORDER = [(dy, dx) for dx in (1, 0, 2) for dy in range(3)]


def _emit(ctx: ExitStack, tc: "tile.TileContext", nc, xq, xb, wvdr_d, wqT_d,
          wks_d, inv_d, inv64_d, bias2_d, out,
          parts=("scores", "conv", "fullevict"), prep_state=None):
    if prep_state is None:
        prep_state = _emit_prep(ctx, tc, nc, wvdr_d, wqT_d, wks_d, inv_d,
                                inv64_d, bias2_d)
    return _emit_main(ctx, tc, nc, xq, xb, out, parts, prep_state)


def _emit_prep(ctx, tc, nc, wvdr_d, wqT_d, wks_d, inv_d, inv64_d, bias2_d):
    """Weights arrive pre-transposed/cast from the host; just stage them."""
    consts = ctx.enter_context(tc.tile_pool(name="consts", bufs=1))
    ones_col = consts.tile([P, 1], FP32, tag="ones")
    nc.vector.memset(ones_col[:], 1.0)

    inv_t, inv64_t, bias2_t = [], [], []
    for mt in range(MT):
        iv = consts.tile([P, 1], FP32, tag=f"inv{mt}")
        nc.sync.dma_start(iv[:], inv_d[mt * P:(mt + 1) * P])
        inv_t.append(iv)
        iv64 = consts.tile([P, 1], FP32, tag=f"inv64{mt}")
        nc.sync.dma_start(iv64[:], inv64_d[mt * P:(mt + 1) * P])
        inv64_t.append(iv64)
        b2 = consts.tile([P, 1], FP32, tag=f"b2{mt}")
        nc.sync.dma_start(b2[:], bias2_d[mt * P:(mt + 1) * P])
        bias2_t.append(b2)

    wT_pool = ctx.enter_context(tc.tile_pool(name="wT", bufs=1))
    wv = wT_pool.tile([P, NTAP * KT * C], FP8, tag="wvdr", name="wvdr")
    nc.sync.dma_start(wv[:], wvdr_d)
    wqT = [wT_pool.tile([P, NTAP * C], BF16, tag=f"wqT{kt}", name=f"wqT{kt}")
           for kt in range(KT)]
    wks = [wT_pool.tile([P, NTAP], BF16, tag=f"wks{kt}", name=f"wks{kt}")
           for kt in range(KT)]
    for kt in range(KT):
        nc.sync.dma_start(wqT[kt][:], wqT_d[kt])
        nc.sync.dma_start(wks[kt][:], wks_d[kt])

    return (consts, wv, wqT, wks, ones_col, inv_t, inv64_t, bias2_t)


def _emit_main(ctx, tc, nc, xq, xb, out, parts, prep_state):
    (consts, wv, wqT, wks, ones_col, inv_t, inv64_t, bias2_t) = prep_state
    wv3 = wv[:].rearrange("p (t k o) -> p t k o", t=NTAP, k=KT)

    xq_pool = ctx.enter_context(tc.tile_pool(name="xq", bufs=2))
    xb_pool = ctx.enter_context(tc.tile_pool(name="xb", bufs=2 * KT))
    agg_pool = ctx.enter_context(tc.tile_pool(name="agg", bufs=2 * KT))
    small = ctx.enter_context(tc.tile_pool(name="small", bufs=2))
    ev_pool = ctx.enter_context(tc.tile_pool(name="ev", bufs=3))
    qk_psum = ctx.enter_context(tc.tile_pool(name="qk_psum", bufs=1, space="PSUM"))
    misc_psum = ctx.enter_context(tc.tile_pool(name="misc_psum", bufs=1, space="PSUM"))
    fv_psum = ctx.enter_context(tc.tile_pool(name="fv_psum", bufs=5, space="PSUM"))

    def conv_group(xq5, mt, st):
        y0 = st * ROWS_PER_ST
        pv = fv_psum.tile([P, NF], FP32, tag="fv")
        for i, (dy, dx) in enumerate(TAP_ORDER):
            tap = dy * 3 + dx
            nc.tensor.matmul(
                pv[:], wv3[:, tap, :, mt * P:mt * P + P],
                xq5[:, dx, :, (y0 + dy) * W:(y0 + dy) * W + NF],
                start=(i == 0), stop=(i == NTAP - 1), perf_mode=DR)
        return pv

    def evict_group(b, xbt, pv, s1, mt, st):
        if "fullevict" in parts:
            at = ev_pool.tile([P, NF], FP32, tag="A")
            nc.scalar.activation(
                at[:], xbt[mt][:, st * NF:(st + 1) * NF], ACTF.Identity,
                bias=bias2_t[mt][:], scale=inv_t[mt][:])
            rt = ev_pool.tile([P, NF], FP32, tag="r")
            nc.vector.scalar_tensor_tensor(
                rt[:], pv[:], s1[mt][:], at[:], op0=ALU.mult, op1=ALU.add)
            o_t = ev_pool.tile([P, NF], FP32, tag="o")
            nc.vector.tensor_scalar_max(o_t[:], rt[:], 0.0)
        else:
            o_t = ev_pool.tile([P, NF], FP32, tag="o")
            nc.vector.tensor_copy(o_t[:], pv[:])
        nc.sync.dma_start(
            out[b, mt * P:(mt + 1) * P].rearrange(
                "c h w -> c (h w)")[:, st * NF:(st + 1) * NF],
            o_t[:])

    for b in range(BPC):
        # ---- input staging: dx=1 plane first (conv + colsums), then 0, 2 ---
        xqt = xq_pool.tile([P, NDX * DXP], FP8, tag="xq")
        for dx in (1, 0, 2):
            nc.sync.dma_start(xqt[:, dx * DXP:(dx + 1) * DXP],
                              xq[b, :, dx * DXP:(dx + 1) * DXP])
        xq5 = xqt[:].rearrange("p (d k f) -> p d k f", d=NDX, k=KT)
        xq_r = xqt[:].rearrange("p (d k r c) -> p d k r c", d=NDX, k=KT, c=W)
        xq_t = xqt[:].rearrange("p (d k r c) -> p d k c r", d=NDX, k=KT, c=W)
        xbt = []
        for kt in range(KT):
            t = xb_pool.tile([P, H * W], BF16, tag="xb")
            nc.sync.dma_start(t[:], xb[b, kt])
            xbt.append(t)

        # ---- column sums + shifted-window aggregates (DVE, from fp8 img) ---
        aggs = []
        if "scores" in parts:
            for kt in range(KT):
                cs = small.tile([P, W], FP32, tag="cs")
                nc.vector.tensor_reduce(
                    cs[:], xq_t[:, 1, kt, :, 1:H + 1], axis=AX.X, op=ALU.add)
                ag = agg_pool.tile([P, 3 * PW], BF16, tag="agg")
                a3 = ag[:].rearrange("p (a c) -> p a c", c=PW)
                nc.vector.memset(a3[:, :, 0], 0.0)
                nc.vector.memset(a3[:, :, PW - 1], 0.0)
                # dy=0 row-window is rows -1..H-2: colsum - bottom row
                nc.vector.tensor_sub(a3[:, 0, 1:W + 1], cs[:],
                                     xq_r[:, 1, kt, H, :])
                nc.vector.tensor_copy(a3[:, 1, 1:W + 1], cs[:])
                # dy=2 row-window is rows 1..H: colsum - top row
                nc.vector.tensor_sub(a3[:, 2, 1:W + 1], cs[:],
                                     xq_r[:, 1, kt, 1, :])
                aggs.append(ag)

        held = []  # (pv, mt, st) conv groups awaiting s1
        if "conv" in parts:
            held.append((conv_group(xq5, 0, 0), 0, 0))
            held.append((conv_group(xq5, 0, 1), 0, 1))

        if "scores" not in parts:
            s1 = inv64_t
        else:
            # Transposed layout: qT[w, c] and kT[w, 1] accumulate on PE, then
            # f_scores row = kT^T @ qT in a single matvec.
            qT = qk_psum.tile([W, C], FP32, tag="qk")
            idx = 0
            for kt in range(KT):
                a3 = aggs[kt][:].rearrange("p (a c) -> p a c", c=PW)
                for tap in range(NTAP):
                    dy, dx = divmod(tap, 3)
                    nc.tensor.matmul(
                        qT[:], a3[:, dy, dx:dx + W],
                        wqT[kt][:, tap * C:(tap + 1) * C],
                        start=(idx == 0), stop=(idx == KT * NTAP - 1))
                    idx += 1
            kTp = misc_psum.tile([W, 1], FP32, tag="stp")
            idx = 0
            for kt in range(KT):
                a3 = aggs[kt][:].rearrange("p (a c) -> p a c", c=PW)
                for tap in range(NTAP):
                    dy, dx = divmod(tap, 3)
                    nc.tensor.matmul(
                        kTp[:], a3[:, dy, dx:dx + W], wks[kt][:, tap:tap + 1],
                        start=(idx == 0), stop=(idx == KT * NTAP - 1))
                    idx += 1
            qT_sb = small.tile([W, C], FP32, tag="qTsb")
            nc.vector.tensor_copy(qT_sb[:], qT[:])
            kT_sb = small.tile([W, 1], FP32, tag="kTsb")
            nc.vector.tensor_copy(kT_sb[:], kTp[:])
            fsrow = misc_psum.tile([1, C], FP32, tag="fsrow")
            nc.tensor.matmul(fsrow[:], kT_sb[:], qT_sb[:],
                             start=True, stop=True)

            # one more conv group while the softmax chain runs
            if "conv" in parts:
                held.append((conv_group(xq5, 0, 2), 0, 2))

            mx = small.tile([1, 1], FP32, tag="mx")
            nc.vector.tensor_reduce(mx[:], fsrow[:], axis=AX.X, op=ALU.max)
            mxs = small.tile([1, 1], FP32, tag="mxs")
            nc.vector.tensor_scalar_mul(mxs[:], mx[:], -float(SCORE_SCALE))
            es = small.tile([1, C], FP32, tag="es")
            nc.scalar.activation(es[:], fsrow[:], ACTF.Exp,
                                 bias=mxs[:], scale=float(SCORE_SCALE))
            ssum = small.tile([1, 1], FP32, tag="ssum")
            nc.vector.tensor_reduce(ssum[:], es[:], axis=AX.X, op=ALU.add)
            rs = small.tile([1, 1], FP32, tag="rs")
            nc.vector.reciprocal(rs[:], ssum[:])
            srow = small.tile([1, C], FP32, tag="srow")
            nc.vector.tensor_scalar_mul(srow[:], es[:], rs[:])

            # scores back to [128,1] per mtile (K=1 matmul), fold in BN
            # inv and the 1/WSCALE fp8 weight prescale
            s1 = []
            for mt in range(MT):
                stp = misc_psum.tile([P, 1], FP32, tag="stp")
                nc.tensor.matmul(stp[:], srow[:, mt * P:(mt + 1) * P],
                                 ones_col[0:1, 0:1], start=True, stop=True)
                t = small.tile([P, 1], FP32, tag=f"s1{mt}")
                nc.vector.tensor_mul(t[:], stp[:], inv64_t[mt][:])
                s1.append(t)

        if "conv" not in parts:
            continue
        for pv, mt, st in held:
            evict_group(b, xbt, pv, s1, mt, st)
        done = {(mt, st) for _, mt, st in held}
        for mt in range(MT):
            for st in range(ST):
                if (mt, st) in done:
                    continue
                pv = conv_group(xq5, mt, st)
                evict_group(b, xbt, pv, s1, mt, st)


def build_nc(repeat: int = 1, loop_n: int | None = None,
             parts=("scores", "conv", "fullevict"), hoist_prep: bool = False):
    nc = bacc.Bacc("TRN2", target_bir_lowering=False, debug=False,
                   num_devices=NCORES)
    xq = nc.dram_tensor("xq", [BPC, P, NDX * DXP], FP8,
                        kind="ExternalInput").ap()
    xb = nc.dram_tensor("xb", [BPC, KT, P, H * W], BF16,
                        kind="ExternalInput").ap()
    wvdr_d = nc.dram_tensor("wvdr", [P, NTAP * KT * C], FP8,
                            kind="ExternalInput").ap()
    wqT_d = nc.dram_tensor("wqT", [KT, P, NTAP * C], BF16,
                           kind="ExternalInput").ap()
    wks_d = nc.dram_tensor("wks", [KT, P, NTAP], BF16,
                           kind="ExternalInput").ap()
    inv_d = nc.dram_tensor("inv", [C], FP32, kind="ExternalInput").ap()
    inv64_d = nc.dram_tensor("inv64", [C], FP32, kind="ExternalInput").ap()
    bias2_d = nc.dram_tensor("bias2", [C], FP32, kind="ExternalInput").ap()
    out = nc.dram_tensor("out", [BPC, C, H, W], FP32, kind="ExternalOutput").ap()
    with tile.TileContext(nc) as tc, ExitStack() as ctx:
        prep_state = None
        if hoist_prep:
            prep_state = _emit_prep(ctx, tc, nc, wvdr_d, wqT_d, wks_d,
                                    inv_d, inv64_d, bias2_d)
        if loop_n is not None:
            with tc.For_i(0, loop_n, 1,
                          hint_engines=(mybir.EngineType.PE,)):
                with ExitStack() as rep_ctx:
                    _emit(rep_ctx, tc, nc, xq, xb, wvdr_d, wqT_d, wks_d,
                          inv_d, inv64_d, bias2_d, out, parts=parts,
                          prep_state=prep_state)
        else:
            for _ in range(repeat):
                with ExitStack() as rep_ctx:
                    _emit(rep_ctx, tc, nc, xq, xb, wvdr_d, wqT_d, wks_d,
                          inv_d, inv64_d, bias2_d, out, parts=parts,
                          prep_state=prep_state)
    nc.compile()
    return nc


_NC_CACHE = None


def _get_nc():
    global _NC_CACHE
    if _NC_CACHE is None:
        _NC_CACHE = build_nc()
    return _NC_CACHE


def make_in_maps(inputs: dict) -> list:
    import ml_dtypes
    FP8NP = ml_dtypes.float8_e4m3
    f32 = lambda k: np.ascontiguousarray(np.asarray(inputs[k], np.float32))
    wq, wk, wv = f32("wq"), f32("wk"), f32("wv")
    gamma, beta = f32("gamma"), f32("beta")
    rmean, rvar = f32("running_mean"), f32("running_var")

    inv = (gamma / np.sqrt(rvar + np.float32(EPS))).astype(np.float32)
    bias2 = (beta - rmean * inv).astype(np.float32)
    inv64 = (inv / np.float32(WSCALE)).astype(np.float32)

    # wv -> [i=128, tap, kt, o] fp8, pre-scaled by WSCALE
    a = wv.reshape(C, KT, P, NTAP)             # o, kt, i, tap
    a = a.transpose(2, 3, 1, 0)                # i, tap, kt, o
    wvdr = np.ascontiguousarray(
        (a * np.float32(WSCALE)).reshape(P, NTAP * KT * C).astype(FP8NP))

    # wq -> per k-tile [i=128, (tap, o)] bf16
    aq = wq.reshape(C, KT, P, NTAP).transpose(1, 2, 3, 0)
    wqT = np.ascontiguousarray(
        aq.reshape(KT, P, NTAP * C).astype(ml_dtypes.bfloat16))
    wks = np.ascontiguousarray(
        wk.sum(axis=0).reshape(KT, P, NTAP).astype(ml_dtypes.bfloat16))

    xfull = np.ascontiguousarray(np.asarray(inputs["x"], dtype=np.float32))
    xr = xfull.reshape(B, KT, P, H, W)
    # fp8 dx-shifted padded planes: [b, i, dx, kt, row(66), col(64)]
    xpad = np.zeros((B, P, KT, PH, PW), dtype=np.float32)
    xpad[:, :, :, 1:H + 1, 1:W + 1] = xr.transpose(0, 2, 1, 3, 4)
    xq_full = np.empty((B, P, NDX, KT, PH, W), dtype=FP8NP)
    for dx in range(NDX):
        xq_full[:, :, dx] = xpad[:, :, :, :, dx:dx + W].astype(FP8NP)
    xq_full = xq_full.reshape(B, P, NDX * DXP)
    # bf16 residual copy, [b, kt, i, h*w]
    xb_full = np.ascontiguousarray(
        xr.reshape(B, KT, P, H * W).astype(ml_dtypes.bfloat16))

    rep = {"wvdr": wvdr, "wqT": wqT, "wks": wks,
           "inv": inv, "inv64": inv64, "bias2": bias2}
    in_maps = []
    for c in range(NCORES):
        m = dict(rep)
        m["xq"] = np.ascontiguousarray(xq_full[c * BPC:(c + 1) * BPC])
        m["xb"] = np.ascontiguousarray(xb_full[c * BPC:(c + 1) * BPC])
        in_maps.append(m)
    return in_maps


def kernel(**inputs) -> np.ndarray:
    import time
    nc = _get_nc()
    in_maps = make_in_maps(inputs)
    last_err = None
    for attempt in range(3):
        try:
            res = bass_utils.run_bass_kernel_spmd(
                nc, in_maps, core_ids=list(range(NCORES)))
            return np.concatenate(
                [res.results[c]["out"] for c in range(NCORES)],
                axis=0).astype(np.float32)
        except Exception as e:  # transient device/tunnel hiccups
            last_err = e
            time.sleep(3)
    raise last_err
